# revision 4
# baseline (speedup 1.0000x reference)
"""Trainium2 Bass kernel for nn_AttentionCropLayer — v2.

Math (validated against the jax reference in the v1 kernel):
  out[c] = R^T X[c] C per sample, with
    R[i,j] = mrow[i]*hat(i - sr[j]),  C[m,b] = mcol[m]*hat(m - sc[b]),
    hat(d) = relu(1 - |d|).

v2 redesign vs v1 (371us baseline):
  * Input: indirect SWDGE gather of only the 60-row crop window per
    (sample, channel) — 9.95MB instead of 17.9MB per core, one contiguous
    25.9KB DRAM run per descriptor instead of 432B runs (full DMA rate).
    Window start s_q = min(4*floor(w_off/4), 48) covers [w_off, w_end) since
    the box is at most 56 rows tall.
  * Hat matrices built per 16-sample slab directly in transposed layout
    (row index on partitions) from rank-1/rank-2 PE broadcasts of per-sample
    scalars — no per-sample PE transposes, no per-sample ACT relus.
  * mm1 runs in f32 (x used straight from the gather, no bf16 convert op);
    mrow folds into the rt build, mcol into the ct build.
  * PSUM->SBUF copies grouped 2 samples at a time; t1 on Pool, o on ACT.
  * Output written bf16 into a [3, 108, S, 108] DRAM layout so DMA runs are
    3456B contiguous (full rate); host transposes back and upcasts.
  * DMA split across 3 queues: SWDGE/Pool (in), ACT HWDGE (out), SP (setup).
"""
import numpy as np
import ml_dtypes

import concourse.bass as bass
import concourse.tile as tile
from concourse import mybir
from concourse.alu_op_type import AluOpType as Op

F32 = mybir.dt.float32
BF16 = mybir.dt.bfloat16
I32 = mybir.dt.int32
AF = mybir.ActivationFunctionType
P = 108
WIN = 60          # gathered window rows per sample
N_CORES = 8
S = 128           # samples per core
SL = 16           # slab size
NG = SL // 2      # 2-sample PSUM groups per slab

_ctr = [0]


def _split_multi_waits(nc):
    """This container's walrus accepts at most ONE sync-wait per instruction
    (none on Drain). Move excess waits onto preceding same-engine no-ops."""
    moved = 0
    for func in nc.m.functions:
        for blk in func.blocks:
            out_insts = []
            changed = False
            for inst in blk.instructions:
                si = inst.sync_info
                waits = list(si.on_wait) if (si and si.on_wait) else []
                limit = 0 if inst.opcode == "Drain" else 1
                if len(waits) > limit:
                    keep, excess = waits[:limit], waits[limit:]
                    for w in excess:
                        _ctr[0] += 1
                        nop = mybir.InstNoOp(
                            name=f"waitsplit-{_ctr[0]}",
                            sync_info=mybir.SyncInfo(on_wait=[w], on_update=[]),
                            bass_nofuse=True,
                            engine=inst.engine,
                        )
                        out_insts.append(nop)
                        moved += 1
                    upd = list(si.on_update) if si.on_update else []
                    inst.sync_info = mybir.SyncInfo(on_wait=keep, on_update=upd)
                    changed = True
                out_insts.append(inst)
            if changed:
                try:
                    blk.instructions = out_insts
                except Exception:
                    blk.clear_instructions()
                    for i in out_insts:
                        blk.add_instruction(i)
    return moved


GATHER = False  # indirect window gather: HW SWDGE semantics differ from the
                # interp (one offset per out partition), so disabled for now


def _build(detect_races=True, sim_gather=True, gather=GATHER):
    win = WIN if gather else P
    nslabs = S // SL
    nc = bass.Bass(detect_race_conditions=detect_races)
    images = nc.declare_dram_parameter("images", [S, 3, P, P], F32, isOutput=False)
    if not sim_gather:
        # sim-only bypass: host-pregathered windows (CoreSim cannot track the
        # dynamic-AP indirect DMA precisely)
        xwin_d = nc.declare_dram_parameter("xwin", [nslabs, win, SL * 3, P],
                                           F32, isOutput=False)
    locs = nc.declare_dram_parameter("locs", [S, 3], F32, isOutput=False)
    iota_d = nc.declare_dram_parameter("iota", [128, P], F32, isOutput=False)
    iotap_d = nc.declare_dram_parameter("iotap", [128, 1], F32, isOutput=False)
    ones1_d = nc.declare_dram_parameter("ones1", [1, 128], F32, isOutput=False)
    idf_d = nc.declare_dram_parameter("idf", [128, 128], F32, isOutput=False)
    baseC_d = nc.declare_dram_parameter("baseC", [S, 3], F32, isOutput=False)
    # output in [c, j, s, b] layout so out-DMA runs are (sl, b) contiguous
    out = nc.declare_dram_parameter("out", [3, P, S, P], BF16, isOutput=True)

    images_rows = images.rearrange("s c i k -> (s c i) k")

    with tile.TileContext(nc) as tc:
        with (
            tc.tile_pool(name="consts", bufs=1) as consts,
            tc.tile_pool(name="setup", bufs=1) as setup,
        ):
            iota = consts.tile([128, P], F32)
            nc.sync.dma_start(out=iota, in_=iota_d[:, :])
            iotap = consts.tile([128, 1], F32)
            nc.sync.dma_start(out=iotap, in_=iotap_d[:, :])
            ones1 = consts.tile([1, 128], F32)
            nc.sync.dma_start(out=ones1, in_=ones1_d[:, :])
            idf = consts.tile([128, 128], F32)
            nc.sync.dma_start(out=idf, in_=idf_d[:, :])
            baseC = consts.tile([S, 3], F32)
            nc.sync.dma_start(out=baseC, in_=baseC_d[:, :])

            lt = setup.tile([S, 3], F32)
            nc.sync.dma_start(out=lt, in_=locs[:, :])

            def col(t, j):
                return t[:, j:j + 1]

            # trunc(m*l + 0.5) == RNE-convert(m*l)
            tx = setup.tile([S, 1], F32)
            ty = setup.tile([S, 1], F32)
            tlh = setup.tile([S, 1], F32)
            for j, m, t in ((0, 27.0, tx), (1, 27.0, ty), (2, 7.0, tlh)):
                v = setup.tile([S, 1], F32, tag="v_scaled")
                nc.vector.tensor_scalar(v, col(lt, j), m, None, Op.mult)
                vi = setup.tile([S, 1], I32, tag="v_int")
                nc.vector.tensor_copy(vi, v)
                nc.vector.tensor_copy(t, vi)

            # w_off = tx - tlh + 33 ; w_end = min(tx + tlh + 75, 108)
            w_off = setup.tile([S, 1], F32)
            nc.vector.scalar_tensor_tensor(w_off, tx, 33.0, tlh, Op.add, Op.subtract)
            w_end = setup.tile([S, 1], F32)
            nc.vector.scalar_tensor_tensor(w_end, tx, 75.0, tlh, Op.add, Op.add)
            nc.vector.tensor_scalar(w_end, w_end, 108.0, None, Op.min)
            h_off = setup.tile([S, 1], F32)
            nc.vector.scalar_tensor_tensor(h_off, ty, 33.0, tlh, Op.add, Op.subtract)
            h_end = setup.tile([S, 1], F32)
            nc.vector.scalar_tensor_tensor(h_end, ty, 75.0, tlh, Op.add, Op.add)
            nc.vector.tensor_scalar(h_end, h_end, 108.0, None, Op.min)

            # window start: s_q = min(4*floor(w_off/4), 48)
            # floor(w_off/4) for integer w_off == RNE(w_off*0.25 - 0.375)
            s_q = setup.tile([S, 1], F32)
            if gather:
                q0 = setup.tile([S, 1], F32)
                nc.vector.tensor_scalar(q0, w_off, 0.25, -0.375, Op.mult, Op.add)
                q0i = setup.tile([S, 1], I32)
                nc.vector.tensor_copy(q0i, q0)
                nc.vector.tensor_copy(q0, q0i)
                nc.vector.tensor_scalar(s_q, q0, 4.0, 48.0, Op.mult, Op.min)
            else:
                nc.vector.memset(s_q, 0.0)

            a_r = setup.tile([S, 1], F32)   # window-coord row start of box
            nc.vector.tensor_sub(a_r, w_off, s_q)
            b_r = setup.tile([S, 1], F32)   # window-coord row end of box
            nc.vector.tensor_sub(b_r, w_end, s_q)

            # negated slopes and starts (for rank-1/2 broadcast matmuls)
            nslope_r = setup.tile([S, 1], F32)
            nc.vector.scalar_tensor_tensor(nslope_r, w_end, -1.0, w_off,
                                           Op.add, Op.subtract)
            nc.vector.tensor_scalar(nslope_r, nslope_r, -1.0 / 107.0, None, Op.mult)
            nslope_c = setup.tile([S, 1], F32)
            nc.vector.scalar_tensor_tensor(nslope_c, h_end, -1.0, h_off,
                                           Op.add, Op.subtract)
            nc.vector.tensor_scalar(nslope_c, nslope_c, -1.0 / 107.0, None, Op.mult)
            na_r = setup.tile([S, 1], F32)
            nc.vector.tensor_scalar(na_r, a_r, -1.0, None, Op.mult)
            na_c = setup.tile([S, 1], F32)
            nc.vector.tensor_scalar(na_c, h_off, -1.0, None, Op.mult)

            # gather offsets (global row index of window start, per (s, c))
            offs_f = setup.tile([S, 3], F32)
            nc.vector.tensor_scalar(offs_f, baseC, s_q, None, Op.add)
            offs = setup.tile([S, 3], I32)
            nc.vector.tensor_copy(offs, offs_f)

            # masks: mrow in window coords [S, WIN], mcol in full coords [S, P]
            def sigpair(dst, n, off_t, end_t, tag):
                b1 = setup.tile([S, 1], F32, tag=f"{tag}_b1")
                nc.vector.tensor_scalar(b1, off_t, -10.0, None, Op.mult)
                b2 = setup.tile([S, 1], F32, tag=f"{tag}_b2")
                nc.vector.tensor_scalar(b2, end_t, -10.0, None, Op.mult)
                s1 = setup.tile([S, n], F32, tag=f"{tag}_s1")
                nc.scalar.activation(s1, iota[:S, :n], AF.Sigmoid, bias=b1, scale=10.0)
                s2 = setup.tile([S, n], F32, tag=f"{tag}_s2")
                nc.scalar.activation(s2, iota[:S, :n], AF.Sigmoid, bias=b2, scale=10.0)
                nc.vector.tensor_sub(dst, s1, s2)

            mrow_w = setup.tile([S, win], F32)
            sigpair(mrow_w, win, a_r, b_r, "mr")
            mcol_f = setup.tile([S, P], F32)
            sigpair(mcol_f, P, h_off, h_end, "mc")

            mrowT = setup.tile([win, S], F32)
            mcolT = setup.tile([P, S], F32)
            u_rB = setup.tile([128, S], F32)
            u_cB = setup.tile([128, S], F32)
            nsl_rB = setup.tile([128, S], F32)
            nsl_cB = setup.tile([128, S], F32)

            with tc.tile_pool(name="setup_ps", bufs=2, space="PSUM") as setup_ps:
                mT_ps = setup_ps.tile([win, S], F32, tag="ps_small")
                nc.tensor.transpose(mT_ps, mrow_w, idf[:S, :S])
                nc.vector.tensor_copy(mrowT, mT_ps)
                mcT_ps = setup_ps.tile([P, S], F32, tag="ps_small")
                nc.tensor.transpose(mcT_ps, mcol_f, idf[:S, :S])
                nc.vector.tensor_copy(mcolT, mcT_ps)

                # per-sample scalars as [1, S] rows (transposes), then rank-1
                # broadcasts over 128 partitions; the per-partition index term
                # of u is added from the iotap column during the PSUM copy.
                for src, dst, add_iotap in (
                    (na_r, u_rB, True), (na_c, u_cB, True),
                    (nslope_r, nsl_rB, False), (nslope_c, nsl_cB, False),
                ):
                    rT_ps = setup_ps.tile([1, S], F32, tag="ps_row")
                    nc.tensor.transpose(rT_ps, src, idf[:S, :S])
                    row = setup.tile([1, S], F32, tag="row_sb")
                    nc.vector.tensor_copy(row, rT_ps)
                    bps = setup_ps.tile([128, S], F32, tag="ps_big")
                    nc.tensor.matmul(bps, ones1, row, start=True, stop=True)
                    if add_iotap:
                        nc.vector.tensor_scalar(dst, bps, iotap, None, Op.add)
                    else:
                        nc.vector.tensor_copy(dst, bps)

            with (
                tc.tile_pool(name="xpool", bufs=3) as xpool,
                tc.tile_pool(name="hpool", bufs=2) as hpool,
                tc.tile_pool(name="rpool", bufs=2) as rpool,
                tc.tile_pool(name="opool", bufs=2) as opool,
                tc.tile_pool(name="samp", bufs=4) as samp,
                tc.tile_pool(name="ps_t1", bufs=2, space="PSUM") as ps_t1,
                tc.tile_pool(name="ps_o", bufs=2, space="PSUM") as ps_o,
            ):
                for t in range(nslabs):
                    s0 = t * SL
                    if gather:
                        # gather win-row windows: one descriptor per (s, chan)
                        x = xpool.tile([win, SL * 3, P], F32, tag="x")
                        if sim_gather:
                            nc.gpsimd.indirect_dma_start(
                                out=x.transpose([1, 0, 2]),
                                out_offset=None,
                                in_=images_rows[:, :],
                                in_offset=bass.IndirectOffsetOnAxis(
                                    ap=offs[s0:s0 + SL, :], axis=0),
                            )
                        else:
                            nc.gpsimd.dma_start(out=x, in_=xwin_d[t])
                        xs = lambda sl, c: x[:, sl * 3 + c]
                    else:
                        # balance the two HWDGE queues: SP carries 5 input
                        # slabs (~62us), ACT carries 3 input slabs + all
                        # output (~62us); SEQ frees at HWDGE handoff so the
                        # issuing engines don't stall on the transfers
                        x = xpool.tile([P, SL, 3, P], F32, tag="x")
                        in_eng = nc.scalar if t in (1, 4, 7) else nc.sync
                        in_eng.dma_start(
                            out=x,
                            in_=images[s0:s0 + SL, :, :, :].transpose([2, 0, 1, 3]))
                        xs = lambda sl, c: x[:, sl, c]

                    # hat builds, transposed layout, slab granularity.
                    # row chain on DVE, column chain on Pool (engine balance);
                    # abs/relu on ACT either way.
                    def hat_build(n, u_t, nsl_t, scr_tag, eng):
                        scr = hpool.tile([n, SL, P], F32, tag=scr_tag)
                        iota_b = iota[:n, :P].unsqueeze(1).broadcast_to([n, SL, P])
                        nsl_b = nsl_t[:n, s0:s0 + SL].unsqueeze(2).broadcast_to(
                            [n, SL, P])
                        u_b = u_t[:n, s0:s0 + SL].unsqueeze(2).broadcast_to(
                            [n, SL, P])
                        eng.tensor_tensor(scr, iota_b, nsl_b, Op.mult)
                        eng.tensor_tensor(scr, scr, u_b, Op.add)
                        nc.scalar.activation(scr, scr, AF.Abs)
                        nc.scalar.activation(scr, scr, AF.Relu, bias=1.0, scale=-1.0)
                        return scr

                    hr = hat_build(win, u_rB, nsl_rB, "hscr_r", nc.vector)
                    rt = rpool.tile([win, SL, P], F32, tag="rt")
                    mrow_b = mrowT[:, s0:s0 + SL].unsqueeze(2).broadcast_to(
                        [win, SL, P])
                    nc.vector.tensor_tensor(rt, hr, mrow_b, Op.mult)

                    hc = hat_build(P, u_cB, nsl_cB, "hscr_c", nc.gpsimd)
                    ct = rpool.tile([P, SL, P], BF16, tag="ct")
                    mcol_b = mcolT[:, s0:s0 + SL].unsqueeze(2).broadcast_to(
                        [P, SL, P])
                    nc.gpsimd.tensor_tensor(ct, hc, mcol_b, Op.mult)

                    o_sb = opool.tile([P, 3, SL, P], BF16, tag="o_sb")

                    for g in range(NG):
                        t1_ps = ps_t1.tile([P, 3, 2, 128], F32, tag="t1")
                        for e in range(2):
                            sl = 2 * g + e
                            for c in range(3):
                                nc.tensor.matmul(t1_ps[:, c, e, :P],
                                                 xs(sl, c), rt[:, sl],
                                                 start=True, stop=True)
                        t1 = samp.tile([P, 3, 2, P], BF16, tag="t1sb")
                        # alternate copy engines by group parity so adjacent
                        # groups' chains don't serialize on one engine
                        if g % 2 == 0:
                            nc.vector.tensor_copy(t1, t1_ps[:, :, :, :P])
                        else:
                            nc.scalar.activation(t1, t1_ps[:, :, :, :P], AF.Copy)

                        o_ps = ps_o.tile([P, 3, 2, 128], F32, tag="o")
                        for e in range(2):
                            sl = 2 * g + e
                            for c in range(3):
                                nc.tensor.matmul(o_ps[:, c, e, :P],
                                                 t1[:, c, e], ct[:, sl],
                                                 start=True, stop=True)
                        if g % 2 == 0:
                            nc.scalar.activation(o_sb[:, :, 2 * g:2 * g + 2, :],
                                                 o_ps[:, :, :, :P], AF.Copy)
                        else:
                            nc.vector.tensor_copy(
                                o_sb[:, :, 2 * g:2 * g + 2, :],
                                o_ps[:, :, :, :P])

                    nc.scalar.dma_start(
                        out=out[:, :, s0:s0 + SL, :].transpose([1, 0, 2, 3]),
                        in_=o_sb)
    return nc


def _host_constants():
    iota = np.tile(np.arange(P, dtype=np.float32), (128, 1))
    iotap = np.arange(128, dtype=np.float32)[:, None]
    ones1 = np.ones((1, 128), dtype=np.float32)
    idf = np.eye(128, dtype=np.float32)
    s = np.arange(S, dtype=np.float32)[:, None]
    c = np.arange(3, dtype=np.float32)[None, :]
    baseC = ((s * 3 + c) * P).astype(np.float32)
    return {"iota": iota, "iotap": iotap, "ones1": ones1, "idf": idf,
            "baseC": baseC}


_cached_nc = None


def _get_nc():
    global _cached_nc
    if _cached_nc is None:
        nc = _build()
        _split_multi_waits(nc)
        _cached_nc = nc
    return _cached_nc


def kernel(images: np.ndarray, locs: np.ndarray) -> np.ndarray:
    from concourse.bass_utils import run_bass_kernel_spmd

    images = np.ascontiguousarray(np.asarray(images, dtype=np.float32))
    locs = np.ascontiguousarray(np.asarray(locs, dtype=np.float32))
    assert images.shape == (N_CORES * S, 3, P, P), images.shape
    assert locs.shape == (N_CORES * S, 3), locs.shape

    nc = _get_nc()
    consts = _host_constants()
    in_maps = [
        {
            "images": images[c * S:(c + 1) * S],
            "locs": locs[c * S:(c + 1) * S],
            **consts,
        }
        for c in range(N_CORES)
    ]
    res = run_bass_kernel_spmd(nc, in_maps, list(range(N_CORES)))
    # out is [3, 108, S, 108] bf16 per core -> [S, 3, 108, 108] f32
    parts = [
        np.asarray(res.results[c]["out"]).transpose(2, 0, 1, 3).astype(np.float32)
        for c in range(N_CORES)
    ]
    return np.concatenate(parts, axis=0)
